# revision 11
# baseline (speedup 1.0000x reference)
"""VQ codebook lookup kernel for Trainium2 (8 NeuronCores, data-parallel).

Computes out[b] = values[argmin_k ||x[b] - keys[k]||] for
x [65536, 512], keys/values [1024, 512] fp32.

Strategy (per core, batch shard of 8192 rows):
  - argmin of distance == argmax of s = 2*x.k - |k|^2.
  - fp32 precision via bf16 hi/lo split, 3 matmul passes
    (hi*hi + hi*lo + lo*hi); the -|k|^2 bias is folded into the same
    PSUM accumulation as a 4th matmul against an all-ones stationary
    operand (bias rows = 3-way bf16 split of -|k|^2).
  - Host prep: transpose x shard to [512, 8192] and split to bf16 hi/lo;
    same for (2*keys)^T.
  - Device: PE matmuls -> DVE MAX8/FIND_INDEX8 straight from PSUM ->
    indirect-DMA gather of values rows -> DMA out.
"""

import numpy as np

_B = 65536
_D = 512
_K = 1024
_NCORES = 8
_BL = _B // _NCORES  # 8192 rows per core
_P = 128
_BBLK = 512          # b columns loaded per DMA
_BT = 128            # b rows per matmul tile (PSUM partition dim)
_DC = _D // _P       # 4 contraction chunks

_cached = None

# If True, fold -|k|^2 into the PE matmul accumulation (costs 2 extra
# N=512 matmuls per b-tile).  If False, subtract it on the vector engine
# fused with the PSUM->SBUF copy.
_BIAS_ON_PE = False


def _build():
    import concourse.mybir as mybir
    from concourse import bacc
    from concourse.bass import IndirectOffsetOnAxis
    from concourse.tile import TileContext

    f32 = mybir.dt.float32
    bf16 = mybir.dt.bfloat16
    u32 = mybir.dt.uint32

    nc = bacc.Bacc("TRN2", target_bir_lowering=False, debug=False,
                   num_devices=_NCORES)
    xTh = nc.dram_tensor("xTh", [_D, _BL], bf16, kind="ExternalInput")
    xTl = nc.dram_tensor("xTl", [_D, _BL], bf16, kind="ExternalInput")
    kTh = nc.dram_tensor("kTh", [_D, _K], bf16, kind="ExternalInput")
    kTl = nc.dram_tensor("kTl", [_D, _K], bf16, kind="ExternalInput")
    biasp = nc.dram_tensor("biasp", [_P, _K], bf16, kind="ExternalInput")
    k2r = nc.dram_tensor("k2r", [_P, _K], f32, kind="ExternalInput")
    vals = nc.dram_tensor("vals", [_K, _D], f32, kind="ExternalInput")
    out = nc.dram_tensor("out", [_BL, _D], f32, kind="ExternalOutput")

    xTh3 = xTh.rearrange("(do p) b -> p do b", p=_P)   # [128, 4, 8192]
    xTl3 = xTl.rearrange("(do p) b -> p do b", p=_P)
    kTh3 = kTh.rearrange("(do p) k -> p do k", p=_P)   # [128, 4, 1024]
    kTl3 = kTl.rearrange("(do p) k -> p do k", p=_P)

    with TileContext(nc) as tc:
        with (
            tc.tile_pool(name="const", bufs=1) as cpool,
            tc.tile_pool(name="xp", bufs=4) as xpool,
            tc.tile_pool(name="sp", bufs=3) as spool,
            tc.tile_pool(name="st", bufs=4) as stpool,
            tc.tile_pool(name="gp", bufs=4) as gpool,
            tc.tile_pool(name="ps", bufs=3, space="PSUM") as pspool,
        ):
    # Const loads go on the Scalar engine's HWDGE queue so they overlap
    # with the x-block loads issued from the Sync engine (descriptor
    # generation serializes per issuing engine).
            kh_sb = cpool.tile([_P, _DC, _K], bf16)
            nc.scalar.dma_start(kh_sb[:], kTh3[:, :, :])
            kl_sb = cpool.tile([_P, _DC, _K], bf16)
            nc.scalar.dma_start(kl_sb[:], kTl3[:, :, :])
            if _BIAS_ON_PE:
                bias_sb = cpool.tile([_P, _K], bf16)
                nc.scalar.dma_start(bias_sb[:], biasp[:, :])
                ones_sb = cpool.tile([_P, _P], bf16)
                nc.vector.memset(ones_sb[:], 1.0)
            else:
                k2_sb = cpool.tile([_P, _K], f32)
                nc.scalar.dma_start(k2_sb[:], k2r[:, :])

            # First block is a single b-tile so the PE starts ~5us sooner;
            # remaining blocks are _BBLK wide.
            blocks = [(0, _BT)]
            off = _BT
            while off < _BL:
                w = min(_BBLK, _BL - off)
                blocks.append((off, w))
                off += w

            for boff, bw in blocks:
                xth = xpool.tile([_P, _DC, _BBLK], bf16, tag="xth")
                xtl = xpool.tile([_P, _DC, _BBLK], bf16, tag="xtl")
                nc.sync.dma_start(xth[:, :, :bw], xTh3[:, :, boff:boff + bw])
                nc.sync.dma_start(xtl[:, :, :bw], xTl3[:, :, boff:boff + bw])

                for sub in range(bw // _BT):
                    bt = boff // _BT + sub
                    bsl = slice(sub * _BT, (sub + 1) * _BT)
                    ps = pspool.tile([_P, _K], f32)
                    if not _BIAS_ON_PE:
                        s = spool.tile([_P, _K], f32)
                    for h in range(2):
                        hsl = slice(h * 512, (h + 1) * 512)
                        po = ps[:, hsl]
                        if _BIAS_ON_PE:
                            nc.tensor.matmul(po, lhsT=ones_sb[:],
                                             rhs=bias_sb[:, hsl],
                                             start=True, stop=False)
                        for dc in range(_DC):
                            nc.tensor.matmul(po, lhsT=xth[:, dc, bsl],
                                             rhs=kh_sb[:, dc, hsl],
                                             start=(not _BIAS_ON_PE and dc == 0),
                                             stop=False)
                            nc.tensor.matmul(po, lhsT=xth[:, dc, bsl],
                                             rhs=kl_sb[:, dc, hsl],
                                             start=False, stop=False)
                        for dc in range(_DC):
                            nc.tensor.matmul(po, lhsT=xtl[:, dc, bsl],
                                             rhs=kh_sb[:, dc, hsl],
                                             start=False, stop=(dc == _DC - 1))
                        if not _BIAS_ON_PE:
                            # s = 2*x.k - |k|^2, fused PSUM->SBUF move
                            nc.vector.tensor_sub(
                                out=s[:, hsl], in0=po, in1=k2_sb[:, hsl])
                    sc = ps if _BIAS_ON_PE else s
                    mx = stpool.tile([_P, 8], f32)
                    nc.vector.max(out=mx[:], in_=sc[:])
                    idx = stpool.tile([_P, 8], u32)
                    nc.vector.max_index(out=idx[:], in_max=mx[:], in_values=sc[:])

                    g = gpool.tile([_P, _D], f32)
                    nc.gpsimd.indirect_dma_start(
                        out=g[:],
                        out_offset=None,
                        in_=vals[:, :],
                        in_offset=IndirectOffsetOnAxis(ap=idx[:, :1], axis=0),
                    )
                    nc.scalar.dma_start(out[bt * _BT:(bt + 1) * _BT, :], g[:])

    nc.compile()
    return nc


def _get_nc():
    global _cached
    if _cached is None:
        _cached = _build()
    return _cached


def _hi_lo(a):
    """Split fp32 array into bf16 hi + bf16 lo with hi + lo ~ a."""
    import ml_dtypes

    hi = a.astype(ml_dtypes.bfloat16)
    lo = (a - hi.astype(np.float32)).astype(ml_dtypes.bfloat16)
    return hi, lo


def _prepare_in_maps(x, keys, values):
    x = np.asarray(x, dtype=np.float32)
    keys = np.asarray(keys, dtype=np.float32)
    values = np.asarray(values, dtype=np.float32)

    import ml_dtypes

    k2T = np.ascontiguousarray((2.0 * keys).T)          # [512, 1024] f32
    kTh, kTl = _hi_lo(k2T)

    # 3-way bf16 split of -|k|^2, padded to 128 partition rows
    b64 = -np.einsum("kd,kd->k", keys.astype(np.float64),
                     keys.astype(np.float64))
    bh = b64.astype(ml_dtypes.bfloat16)
    bm = (b64 - bh.astype(np.float64)).astype(ml_dtypes.bfloat16)
    bl = (b64 - bh.astype(np.float64) - bm.astype(np.float64)).astype(
        ml_dtypes.bfloat16)
    biasp = np.zeros((_P, _K), dtype=ml_dtypes.bfloat16)
    biasp[0], biasp[1], biasp[2] = bh, bm, bl

    k2 = np.einsum("kd,kd->k", keys, keys).astype(np.float32)
    k2r = np.ascontiguousarray(np.broadcast_to(k2, (_P, _K)))

    in_maps = []
    for c in range(_NCORES):
        xs = np.ascontiguousarray(x[c * _BL:(c + 1) * _BL].T)  # [512, 8192]
        xh, xl = _hi_lo(xs)
        in_maps.append({"xTh": xh, "xTl": xl, "kTh": kTh, "kTl": kTl,
                        "biasp": biasp, "k2r": k2r, "vals": values})
    return in_maps


def kernel(x, keys, values):
    from concourse.bass_utils import run_bass_kernel_spmd

    nc = _get_nc()
    in_maps = _prepare_in_maps(x, keys, values)
    res = run_bass_kernel_spmd(nc, in_maps, core_ids=list(range(_NCORES)))
    return np.concatenate([r["out"] for r in res.results], axis=0)
